# revision 21
# baseline (speedup 1.0000x reference)
"""Multi-head attention (B=4, S=2048, D=768, H=12) on 8 Trainium2 cores.

Sharding: 48 (batch, head) pairs split 6-per-core; core c handles batch
c//2, local heads 6*(c%2) .. 6*(c%2)+5.  Each core computes its heads'
probs [6, S, S] (the 100MB/core memory-roofline term) and its ctx
columns [S, 384]; the host reassembles the full outputs.

Pipeline per core (bf16 matmuls, fp32 accumulation):
  hs -> hsT (PE transpose)  ->  Q^T/K^T/V^T head-pair projections
  scores = (qT|1)^T @ (kT|maskrow)  (K=65 matmul folds the mask add)
  exp on ScalarE (scale=1/8, accum_out row sums, no row-max: scores ~N(0,1))
  probs_f32 = exp * (1/sum)  on VectorE -> DMA to HBM
  expT via PE transposes -> ctx^T = sum_k v_chunk^T.T @ expT  -> scale, DMA.
"""

import numpy as np
import ml_dtypes

import concourse.bacc as bacc
import concourse.bass as bass
import concourse.tile as tile
from concourse import mybir
from concourse.masks import make_identity

B, S, D = 4, 2048, 768
H, HD = 12, 64
NCORES = 8
HPC = H * B // NCORES  # 6 heads per core
SCALE = 1.0 / np.sqrt(D)  # weight prescale used by reference setup (unused here)
INV_SQRT_HD = 0.125

F32 = mybir.dt.float32
BF16 = mybir.dt.bfloat16

NQ = S // 128  # 16 query chunks
NC_D = D // 128  # 6 contraction chunks for projections
NK = S // 128  # 16 key chunks

# tunables (overridable for experiments)
import os as _os

TGRP = int(_os.environ.get("K_TGRP", "8"))  # transposes packed per PSUM evict
NORM_POOL_MOD = int(_os.environ.get("K_NORM_POOL_MOD", "0"))  # qi % mod == 0 -> gpsimd
EXP_BUFS = int(_os.environ.get("K_EXP_BUFS", "4"))
PRB_BUFS = int(_os.environ.get("K_PRB_BUFS", "3"))
QK_BUFS = int(_os.environ.get("K_QK_BUFS", "2"))
PSA_BUFS = int(_os.environ.get("K_PSA_BUFS", "2"))
PSB_BUFS = int(_os.environ.get("K_PSB_BUFS", "3"))
PTP_BUFS = int(_os.environ.get("K_PTP_BUFS", "2"))
PSC_BUFS = int(_os.environ.get("K_PSC_BUFS", "1"))


def build_bass():
    nc = bacc.Bacc()
    hs = nc.declare_dram_parameter("hs", [S, D], BF16, isOutput=False)
    w3 = nc.declare_dram_parameter("w3", [D, 3 * HPC * HD], BF16, isOutput=False)
    b3 = nc.declare_dram_parameter("b3", [3 * HPC * HD], F32, isOutput=False)
    maskrow = nc.declare_dram_parameter("maskrow", [S], BF16, isOutput=False)
    probs_out = nc.declare_dram_parameter("probs_out", [HPC, S, S], F32, isOutput=True)
    ctx_out = nc.declare_dram_parameter("ctx_out", [S, HPC * HD], F32, isOutput=True)

    with tile.TileContext(nc) as tc:
        with (
            tc.tile_pool(name="consts", bufs=1) as consts,
            tc.tile_pool(name="psA", bufs=PSA_BUFS, space="PSUM") as psA,
            tc.tile_pool(name="psB", bufs=PSB_BUFS, space="PSUM") as psB,
            tc.tile_pool(name="psC", bufs=PSC_BUFS, space="PSUM") as psC,
            tc.tile_pool(name="qk", bufs=QK_BUFS) as qk,
            tc.tile_pool(name="expp", bufs=EXP_BUFS) as expp,
            tc.tile_pool(name="prbp", bufs=PRB_BUFS) as prbp,
            tc.tile_pool(name="ptp", bufs=PTP_BUFS) as ptp,
            tc.tile_pool(name="stat", bufs=4) as stat,
            tc.tile_pool(name="recp", bufs=12) as recp,
            tc.tile_pool(name="ctxp", bufs=3) as ctxp,
        ):
            ident_bf = consts.tile([128, 128], BF16)
            make_identity(nc, ident_bf)
            ident_f32 = consts.tile([128, 128], F32)
            make_identity(nc, ident_f32)

            # ---- constants: weights, biases, mask ----
            wb = consts.tile([128, NC_D, 3 * HPC * HD], BF16)  # [p, c, n]
            w3r = w3.rearrange("(c p) n -> p c n", p=128)
            for t in range(3):
                nc.sync.dma_start(
                    out=wb[:, :, t * 3 * HD * 2 : (t + 1) * 3 * HD * 2],
                    in_=w3r[:, :, t * 3 * HD * 2 : (t + 1) * 3 * HD * 2],
                )
            bias_sb = consts.tile([128, 9], F32)
            nc.sync.dma_start(out=bias_sb, in_=b3.rearrange("(j p) -> p j", p=128))
            maskc = consts.tile([1, S], BF16)
            nc.sync.dma_start(out=maskc, in_=maskrow[None, :])

            # ---- hs (bf16, natural layout) ----
            hsb = consts.tile([128, NQ, D], BF16)  # hsb[p, si, d] = hs[si*128+p, d]
            hsr = hs.rearrange("(si p) d -> p si d", p=128)
            for si in range(NQ):
                nc.sync.dma_start(out=hsb[:, si, :], in_=hsr[:, si, :])
            hsT = consts.tile([128, NC_D, S], BF16)  # [p, c, s] = hs[s, c*128+p]

            def alloc_pair(pp):
                return dict(
                    qTs=[
                        qk.tile([65, S], BF16, tag=f"qT{i}", name=f"qT{i}_{pp}")
                        for i in range(2)
                    ],
                    kTs=[
                        qk.tile([65, S], BF16, tag=f"kT{i}", name=f"kT{i}_{pp}")
                        for i in range(2)
                    ],
                    vT2=qk.tile([128, S], BF16, tag="vT2", name=f"vT2_{pp}"),
                    v2=qk.tile([128, NK, 128], BF16, tag="v2", name=f"v2_{pp}"),
                )

            def proj_chunks(pp, T):
                """Closures emitting next pair's projections; callable one per qi."""
                chunks = []

                def rows(T=T):
                    for hh2 in range(2):
                        nc.vector.memset(T["qTs"][hh2][64:65, :], 1.0)
                        nc.vector.tensor_copy(out=T["kTs"][hh2][64:65, :], in_=maskc)

                chunks.append(rows)
                for t in range(3):
                    j = t * 3 + pp
                    for sb_i in range(4):
                        def mk(t=t, sb_i=sb_i, j=j, T=T):
                            ps = psB.tile([128, 512], F32, tag="mix", name="projps")
                            for c in range(NC_D):
                                nc.tensor.matmul(
                                    ps,
                                    wb[:, c, j * 128 : (j + 1) * 128],
                                    hsT[:, c, sb_i * 512 : (sb_i + 1) * 512],
                                    start=(c == 0),
                                    stop=(c == NC_D - 1),
                                )
                            sl = slice(sb_i * 512, (sb_i + 1) * 512)
                            if t == 2:
                                nc.vector.tensor_scalar_add(
                                    T["vT2"][:, sl], ps, bias_sb[:, j : j + 1]
                                )
                            else:
                                dst = T["qTs"] if t == 0 else T["kTs"]
                                nc.vector.tensor_scalar_add(
                                    dst[0][0:64, sl], ps[0:64, :], bias_sb[0:64, j : j + 1]
                                )
                                nc.vector.tensor_scalar_add(
                                    dst[1][0:64, sl],
                                    ps[64:128, :],
                                    bias_sb[64:128, j : j + 1],
                                )

                        chunks.append(mk)
                for g in range(4):
                    def mkv(g=g, T=T):
                        pk = psB.tile([128, 4, 128], BF16, tag="mix", name="vtp")
                        for cc in range(4):
                            c = 4 * g + cc
                            nc.tensor.transpose(
                                pk[:, cc, :], T["vT2"][:, c * 128 : (c + 1) * 128], ident_bf
                            )
                        nc.any.tensor_copy(out=T["v2"][:, 4 * g : 4 * g + 4, :], in_=pk)

                    chunks.append(mkv)
                return chunks

            # ---- phase 0: hsT transposes interleaved with pair-0 projections ----
            T = alloc_pair(0)
            p0 = proj_chunks(0, T)
            p0[0]()  # rows init
            for sb in range(4):
                for c in range(NC_D):
                    pk = psB.tile([128, 4, 128], BF16, tag="mix", name="hstp")
                    for u in range(4):
                        si = sb * 4 + u
                        nc.tensor.transpose(
                            pk[:, u, :], hsb[:, si, c * 128 : (c + 1) * 128], ident_bf
                        )
                    nc.any.tensor_copy(out=hsT[:, c, sb * 512 : (sb + 1) * 512], in_=pk)
                for t in range(3):
                    p0[1 + t * 4 + sb]()
            for g in range(4):
                p0[13 + g]()

            # ---- per head-pair, software-pipelined ----
            for pp in range(3):
                if pp < 2:
                    Tn = alloc_pair(pp + 1)
                    pend_proj = list(proj_chunks(pp + 1, Tn))
                else:
                    Tn, pend_proj = None, []

                for hh in range(2):
                    h_local = pp * 2 + hh
                    qT, kT = T["qTs"][hh], T["kTs"][hh]
                    v2 = T["v2"]

                    recs = {}
                    probsT = None
                    pend_ctx = []
                    for qi in range(NQ):
                        jq = qi % 4
                        if jq == 0:
                            probsT = ptp.tile([128, NK, 4, 128], BF16, tag="probsT")
                        acc = stat.tile([128, 2], F32, tag="acc")
                        expq = expp.tile([128, S], BF16, tag="expq")
                        for half in range(2):
                            ps = psA.tile([128, 1024], F32, tag="sc")
                            for nb in range(2):
                                nc.tensor.matmul(
                                    ps[:, nb * 512 : (nb + 1) * 512],
                                    qT[:, qi * 128 : (qi + 1) * 128],
                                    kT[
                                        :,
                                        half * 1024 + nb * 512 : half * 1024
                                        + (nb + 1) * 512,
                                    ],
                                    start=True,
                                    stop=True,
                                )
                            nc.scalar.activation(
                                out=expq[:, half * 1024 : (half + 1) * 1024],
                                in_=ps,
                                func=mybir.ActivationFunctionType.Exp,
                                scale=INV_SQRT_HD,
                                accum_out=acc[:, half : half + 1],
                            )
                        sumt = stat.tile([128, 1], F32, tag="sumt")
                        nc.vector.tensor_add(sumt, acc[:, 0:1], acc[:, 1:2])
                        recip = recp.tile([128, 1], F32, tag="recip")
                        nc.vector.reciprocal(recip, sumt)
                        recs[qi] = recip

                        prb = prbp.tile([128, S], F32, tag="prb")
                        nc.vector.tensor_scalar_mul(prb, expq, recip)
                        nc.sync.dma_start(
                            out=probs_out[h_local, qi * 128 : (qi + 1) * 128, :],
                            in_=prb,
                        )

                        for g in range(NK // TGRP):
                            pk = psB.tile([128, TGRP, 128], BF16, tag="mix", name="ptk")
                            for cc in range(TGRP):
                                c = TGRP * g + cc
                                nc.tensor.transpose(
                                    pk[:, cc, :],
                                    expq[:, c * 128 : (c + 1) * 128],
                                    ident_bf,
                                )
                            nc.any.tensor_copy(
                                out=probsT[:, TGRP * g : TGRP * g + TGRP, jq, :],
                                in_=pk,
                            )

                        if pend_ctx:
                            pend_ctx.pop(0)()
                        if hh == 1 and pend_proj:
                            pend_proj.pop(0)()

                        if jq == 3:
                            qb = qi // 4
                            cps = psC.tile([64, 512], F32, tag="ctx", name="cps")
                            pT = probsT

                            def mkctx(u, cps=cps, pT=pT, v2=v2, hh=hh):
                                def go():
                                    for cc in range(4):
                                        c = 4 * u + cc
                                        nc.tensor.matmul(
                                            cps,
                                            v2[:, c, hh * 64 : (hh + 1) * 64],
                                            pT[:, c, :, :],
                                            start=(c == 0),
                                            stop=(c == NK - 1),
                                        )
                                return go

                            def mkfin(qb=qb, cps=cps, hh=hh, h_local=h_local, rr=dict(recs)):
                                def go():
                                    csb = ctxp.tile([64, 512], F32, tag="csb", name="csb")
                                    nc.any.tensor_copy(out=csb, in_=cps)
                                    for jj in range(4):
                                        tps = psB.tile(
                                            [128, 64], F32, tag="mix", name="tps"
                                        )
                                        nc.tensor.transpose(
                                            tps,
                                            csb[:, jj * 128 : (jj + 1) * 128],
                                            ident_f32[0:64, 0:64],
                                        )
                                        cf = ctxp.tile([128, 64], F32, tag="cf", name="cf")
                                        nc.vector.tensor_scalar_mul(
                                            cf, tps, rr[qb * 4 + jj]
                                        )
                                        r = qb * 4 + jj
                                        nc.sync.dma_start(
                                            out=ctx_out[
                                                r * 128 : (r + 1) * 128,
                                                h_local * 64 : (h_local + 1) * 64,
                                            ],
                                            in_=cf,
                                        )
                                return go

                            for u in range(4):
                                pend_ctx.append(mkctx(u))
                            pend_ctx.append(mkfin())
                    while pend_ctx:
                        pend_ctx.pop(0)()
                while pend_proj:
                    pend_proj.pop(0)()
                T = Tn
    nc.finalize()
    return nc


_NC_CACHE = None


def _get_nc():
    global _NC_CACHE
    if _NC_CACHE is None:
        _NC_CACHE = build_bass()
    return _NC_CACHE


def make_in_maps(hidden_states, attention_mask, Wq, bq, Wk, bk, Wv, bv):
    hidden_states = np.asarray(hidden_states, dtype=np.float32)
    attention_mask = np.asarray(attention_mask, dtype=np.float32)
    Wq, Wk, Wv = (np.asarray(w, dtype=np.float32) for w in (Wq, Wk, Wv))
    bq, bk, bv = (np.asarray(b, dtype=np.float32) for b in (bq, bk, bv))

    in_maps = []
    for core in range(NCORES):
        b = core // 2
        h0 = HPC * (core % 2)
        cols = slice(h0 * HD, (h0 + HPC) * HD)
        w3 = np.ascontiguousarray(
            np.concatenate([Wq[:, cols], Wk[:, cols], Wv[:, cols]], axis=1)
        ).astype(ml_dtypes.bfloat16)
        b3 = np.concatenate([bq[cols], bk[cols], bv[cols]])
        # scores get multiplied by 1/8 inside exp; pre-scale mask add by 8 so
        # the net additive term matches the reference's -10000*(1-mask).
        maskrow = ((1.0 - attention_mask[b]) * (-80000.0)).astype(ml_dtypes.bfloat16)
        in_maps.append(
            {
                "hs": np.ascontiguousarray(hidden_states[b]).astype(ml_dtypes.bfloat16),
                "w3": w3,
                "b3": np.ascontiguousarray(b3),
                "maskrow": maskrow,
            }
        )
    return in_maps


def run(in_maps, **kwargs):
    from concourse.bass_utils import run_bass_kernel_spmd

    nc = _get_nc()
    return run_bass_kernel_spmd(nc, in_maps, core_ids=list(range(NCORES)), **kwargs)


def kernel(hidden_states, attention_mask, Wq, bq, Wk, bk, Wv, bv):
    in_maps = make_in_maps(hidden_states, attention_mask, Wq, bq, Wk, bk, Wv, bv)
    res = run(in_maps)
    ctx = np.empty((B, S, D), dtype=np.float32)
    probs = np.empty((B, H, S, S), dtype=np.float32)
    for core in range(NCORES):
        b = core // 2
        h0 = HPC * (core % 2)
        r = res.results[core]
        probs[b, h0 : h0 + HPC] = r["probs_out"]
        ctx[b, :, h0 * HD : (h0 + HPC) * HD] = r["ctx_out"]
    return ctx, probs


if __name__ == "__main__":
    nc = build_bass()
    print("built ok")


# revision 23
# speedup vs baseline: 218.1948x; 218.1948x over previous
"""Multi-head attention (B=4, S=2048, D=768, H=12) on 8 Trainium2 cores.

Sharding: 48 (batch, head) pairs split 6-per-core; core c handles batch
c//2, local heads 6*(c%2) .. 6*(c%2)+5.  Each core computes its heads'
probs [6, S, S] (the 100MB/core memory-roofline term) and its ctx
columns [S, 384]; the host reassembles the full outputs.

Pipeline per core (bf16 matmuls, fp32 accumulation):
  hs -> hsT (PE transpose)  ->  Q^T/K^T/V^T head-pair projections
  scores = (qT|1)^T @ (kT|maskrow)  (K=65 matmul folds the mask add)
  exp on ScalarE (scale=1/8, accum_out row sums, no row-max: scores ~N(0,1))
  probs_f32 = exp * (1/sum)  on VectorE -> DMA to HBM
  expT via PE transposes -> ctx^T = sum_k v_chunk^T.T @ expT  -> scale, DMA.
"""

import numpy as np
import ml_dtypes

import concourse.bacc as bacc
import concourse.bass as bass
import concourse.tile as tile
from concourse import mybir
from concourse.masks import make_identity

B, S, D = 4, 2048, 768
H, HD = 12, 64
NCORES = 8
HPC = H * B // NCORES  # 6 heads per core
SCALE = 1.0 / np.sqrt(D)  # weight prescale used by reference setup (unused here)
INV_SQRT_HD = 0.125

F32 = mybir.dt.float32
BF16 = mybir.dt.bfloat16

NQ = S // 128  # 16 query chunks
NC_D = D // 128  # 6 contraction chunks for projections
NK = S // 128  # 16 key chunks

# tunables (overridable for experiments)
import os as _os

TGRP = int(_os.environ.get("K_TGRP", "8"))  # transposes packed per PSUM evict
NORM_POOL_MOD = int(_os.environ.get("K_NORM_POOL_MOD", "0"))  # qi % mod == 0 -> gpsimd
EXP_BUFS = int(_os.environ.get("K_EXP_BUFS", "4"))
PRB_BUFS = int(_os.environ.get("K_PRB_BUFS", "3"))
QK_BUFS = int(_os.environ.get("K_QK_BUFS", "2"))
PSA_BUFS = int(_os.environ.get("K_PSA_BUFS", "2"))
PSB_BUFS = int(_os.environ.get("K_PSB_BUFS", "3"))
PTP_BUFS = int(_os.environ.get("K_PTP_BUFS", "2"))
PSC_BUFS = int(_os.environ.get("K_PSC_BUFS", "1"))
REPEAT = int(_os.environ.get("K_REPEAT", "1"))


def build_bass():
    nc = bacc.Bacc()
    hs = nc.declare_dram_parameter("hs", [S, D], BF16, isOutput=False)
    w3 = nc.declare_dram_parameter("w3", [D, 3 * HPC * HD], BF16, isOutput=False)
    b3 = nc.declare_dram_parameter("b3", [3 * HPC * HD], F32, isOutput=False)
    maskrow = nc.declare_dram_parameter("maskrow", [S], BF16, isOutput=False)
    probs_out = nc.declare_dram_parameter("probs_out", [HPC, S, S], F32, isOutput=True)
    ctx_out = nc.declare_dram_parameter("ctx_out", [S, HPC * HD], F32, isOutput=True)

    with tile.TileContext(nc) as tc:
        with (
            tc.tile_pool(name="consts", bufs=1) as consts,
            tc.tile_pool(name="psA", bufs=PSA_BUFS, space="PSUM") as psA,
            tc.tile_pool(name="psB", bufs=PSB_BUFS, space="PSUM") as psB,
            tc.tile_pool(name="psC", bufs=PSC_BUFS, space="PSUM") as psC,
            tc.tile_pool(name="qk", bufs=QK_BUFS) as qk,
            tc.tile_pool(name="expp", bufs=EXP_BUFS) as expp,
            tc.tile_pool(name="prbp", bufs=PRB_BUFS) as prbp,
            tc.tile_pool(name="ptp", bufs=PTP_BUFS) as ptp,
            tc.tile_pool(name="stat", bufs=4) as stat,
            tc.tile_pool(name="recp", bufs=12) as recp,
            tc.tile_pool(name="ctxp", bufs=3) as ctxp,
        ):
            ident_bf = consts.tile([128, 128], BF16)
            make_identity(nc, ident_bf)
            ident_f32 = consts.tile([128, 128], F32)
            make_identity(nc, ident_f32)

            # ---- constants: weights, biases, mask ----
            wb = consts.tile([128, NC_D, 3 * HPC * HD], BF16)  # [p, c, n]
            w3r = w3.rearrange("(c p) n -> p c n", p=128)
            for t in range(3):
                nc.sync.dma_start(
                    out=wb[:, :, t * 3 * HD * 2 : (t + 1) * 3 * HD * 2],
                    in_=w3r[:, :, t * 3 * HD * 2 : (t + 1) * 3 * HD * 2],
                )
            bias_sb = consts.tile([128, 9], F32)
            nc.sync.dma_start(out=bias_sb, in_=b3.rearrange("(j p) -> p j", p=128))
            maskc = consts.tile([1, S], BF16)
            nc.sync.dma_start(out=maskc, in_=maskrow[None, :])

            # ---- hs (bf16, natural layout) ----
            hsb = consts.tile([128, NQ, D], BF16)  # hsb[p, si, d] = hs[si*128+p, d]
            hsr = hs.rearrange("(si p) d -> p si d", p=128)
            for si in range(NQ):
                nc.sync.dma_start(out=hsb[:, si, :], in_=hsr[:, si, :])
            hsT = consts.tile([128, NC_D, S], BF16)  # [p, c, s] = hs[s, c*128+p]

            def alloc_pair(pp):
                return dict(
                    qTs=[
                        qk.tile([65, S], BF16, tag=f"qT{i}", name=f"qT{i}_{pp}")
                        for i in range(2)
                    ],
                    kTs=[
                        qk.tile([65, S], BF16, tag=f"kT{i}", name=f"kT{i}_{pp}")
                        for i in range(2)
                    ],
                    vT2=qk.tile([128, S], BF16, tag="vT2", name=f"vT2_{pp}"),
                    v2=qk.tile([128, NK, 128], BF16, tag="v2", name=f"v2_{pp}"),
                )

            def proj_chunks(pp, T):
                """Closures emitting next pair's projections; callable one per qi."""
                chunks = []

                def rows(T=T):
                    for hh2 in range(2):
                        nc.vector.memset(T["qTs"][hh2][64:65, :], 1.0)
                        nc.vector.tensor_copy(out=T["kTs"][hh2][64:65, :], in_=maskc)

                chunks.append(rows)
                for t in range(3):
                    j = t * 3 + pp
                    for sb_i in range(4):
                        def mk(t=t, sb_i=sb_i, j=j, T=T):
                            ps = psB.tile([128, 512], F32, tag="mix", name="projps")
                            for c in range(NC_D):
                                nc.tensor.matmul(
                                    ps,
                                    wb[:, c, j * 128 : (j + 1) * 128],
                                    hsT[:, c, sb_i * 512 : (sb_i + 1) * 512],
                                    start=(c == 0),
                                    stop=(c == NC_D - 1),
                                )
                            sl = slice(sb_i * 512, (sb_i + 1) * 512)
                            if t == 2:
                                nc.vector.tensor_scalar_add(
                                    T["vT2"][:, sl], ps, bias_sb[:, j : j + 1]
                                )
                            else:
                                dst = T["qTs"] if t == 0 else T["kTs"]
                                nc.vector.tensor_scalar_add(
                                    dst[0][0:64, sl], ps[0:64, :], bias_sb[0:64, j : j + 1]
                                )
                                nc.vector.tensor_scalar_add(
                                    dst[1][0:64, sl],
                                    ps[64:128, :],
                                    bias_sb[64:128, j : j + 1],
                                )

                        chunks.append(mk)
                for g in range(4):
                    def mkv(g=g, T=T):
                        pk = psB.tile([128, 4, 128], BF16, tag="mix", name="vtp")
                        for cc in range(4):
                            c = 4 * g + cc
                            nc.tensor.transpose(
                                pk[:, cc, :], T["vT2"][:, c * 128 : (c + 1) * 128], ident_bf
                            )
                        nc.any.tensor_copy(out=T["v2"][:, 4 * g : 4 * g + 4, :], in_=pk)

                    chunks.append(mkv)
                return chunks

            # ---- phase 0: hsT transposes interleaved with pair-0 projections ----
            T = alloc_pair(0)
            p0 = proj_chunks(0, T)
            p0[0]()  # rows init
            for sb in range(4):
                for c in range(NC_D):
                    pk = psB.tile([128, 4, 128], BF16, tag="mix", name="hstp")
                    for u in range(4):
                        si = sb * 4 + u
                        nc.tensor.transpose(
                            pk[:, u, :], hsb[:, si, c * 128 : (c + 1) * 128], ident_bf
                        )
                    nc.any.tensor_copy(out=hsT[:, c, sb * 512 : (sb + 1) * 512], in_=pk)
                for t in range(3):
                    p0[1 + t * 4 + sb]()
            for g in range(4):
                p0[13 + g]()

            # ---- per head-pair, software-pipelined (REPEAT>1 = benchmark mode) ----
            total_pairs = 3 * REPEAT
            for ip in range(total_pairs):
                pp = ip % 3
                if ip + 1 < total_pairs:
                    Tn = alloc_pair(ip + 1)
                    pend_proj = list(proj_chunks((ip + 1) % 3, Tn))
                else:
                    Tn, pend_proj = None, []

                for hh in range(2):
                    h_local = pp * 2 + hh
                    qT, kT = T["qTs"][hh], T["kTs"][hh]
                    v2 = T["v2"]

                    recs = {}
                    probsT = None
                    pend_ctx = []
                    for qi in range(NQ):
                        jq = qi % 4
                        if jq == 0:
                            probsT = ptp.tile([128, NK, 4, 128], BF16, tag="probsT")
                        acc = stat.tile([128, 2], F32, tag="acc")
                        expq = expp.tile([128, S], BF16, tag="expq")
                        for half in range(2):
                            ps = psA.tile([128, 1024], F32, tag="sc")
                            for nb in range(2):
                                nc.tensor.matmul(
                                    ps[:, nb * 512 : (nb + 1) * 512],
                                    qT[:, qi * 128 : (qi + 1) * 128],
                                    kT[
                                        :,
                                        half * 1024 + nb * 512 : half * 1024
                                        + (nb + 1) * 512,
                                    ],
                                    start=True,
                                    stop=True,
                                )
                            nc.scalar.activation(
                                out=expq[:, half * 1024 : (half + 1) * 1024],
                                in_=ps,
                                func=mybir.ActivationFunctionType.Exp,
                                scale=INV_SQRT_HD,
                                accum_out=acc[:, half : half + 1],
                            )
                        sumt = stat.tile([128, 1], F32, tag="sumt")
                        nc.vector.tensor_add(sumt, acc[:, 0:1], acc[:, 1:2])
                        recip = recp.tile([128, 1], F32, tag="recip")
                        nc.vector.reciprocal(recip, sumt)
                        recs[qi] = recip

                        prb = prbp.tile([128, S], F32, tag="prb")
                        nc.vector.tensor_scalar_mul(prb, expq, recip)
                        nc.sync.dma_start(
                            out=probs_out[h_local, qi * 128 : (qi + 1) * 128, :],
                            in_=prb,
                        )

                        for g in range(NK // TGRP):
                            pk = psB.tile([128, TGRP, 128], BF16, tag="mix", name="ptk")
                            for cc in range(TGRP):
                                c = TGRP * g + cc
                                nc.tensor.transpose(
                                    pk[:, cc, :],
                                    expq[:, c * 128 : (c + 1) * 128],
                                    ident_bf,
                                )
                            nc.any.tensor_copy(
                                out=probsT[:, TGRP * g : TGRP * g + TGRP, jq, :],
                                in_=pk,
                            )

                        if pend_ctx:
                            pend_ctx.pop(0)()
                        if hh == 1 and pend_proj:
                            pend_proj.pop(0)()

                        if jq == 3:
                            qb = qi // 4
                            cps = psC.tile([64, 512], F32, tag="ctx", name="cps")
                            pT = probsT

                            def mkctx(u, cps=cps, pT=pT, v2=v2, hh=hh):
                                def go():
                                    for cc in range(4):
                                        c = 4 * u + cc
                                        nc.tensor.matmul(
                                            cps,
                                            v2[:, c, hh * 64 : (hh + 1) * 64],
                                            pT[:, c, :, :],
                                            start=(c == 0),
                                            stop=(c == NK - 1),
                                        )
                                return go

                            def mkfin(qb=qb, cps=cps, hh=hh, h_local=h_local, rr=dict(recs)):
                                def go():
                                    csb = ctxp.tile([64, 512], F32, tag="csb", name="csb")
                                    nc.any.tensor_copy(out=csb, in_=cps)
                                    for jj in range(4):
                                        tps = psB.tile(
                                            [128, 64], F32, tag="mix", name="tps"
                                        )
                                        nc.tensor.transpose(
                                            tps,
                                            csb[:, jj * 128 : (jj + 1) * 128],
                                            ident_f32[0:64, 0:64],
                                        )
                                        cf = ctxp.tile([128, 64], F32, tag="cf", name="cf")
                                        nc.vector.tensor_scalar_mul(
                                            cf, tps, rr[qb * 4 + jj]
                                        )
                                        r = qb * 4 + jj
                                        nc.sync.dma_start(
                                            out=ctx_out[
                                                r * 128 : (r + 1) * 128,
                                                h_local * 64 : (h_local + 1) * 64,
                                            ],
                                            in_=cf,
                                        )
                                return go

                            for u in range(4):
                                pend_ctx.append(mkctx(u))
                            pend_ctx.append(mkfin())
                    while pend_ctx:
                        pend_ctx.pop(0)()
                while pend_proj:
                    pend_proj.pop(0)()
                T = Tn
    nc.finalize()
    return nc


_NC_CACHE = None


def _get_nc():
    global _NC_CACHE
    if _NC_CACHE is None:
        _NC_CACHE = build_bass()
    return _NC_CACHE


def make_in_maps(hidden_states, attention_mask, Wq, bq, Wk, bk, Wv, bv):
    hidden_states = np.asarray(hidden_states, dtype=np.float32)
    attention_mask = np.asarray(attention_mask, dtype=np.float32)
    Wq, Wk, Wv = (np.asarray(w, dtype=np.float32) for w in (Wq, Wk, Wv))
    bq, bk, bv = (np.asarray(b, dtype=np.float32) for b in (bq, bk, bv))

    in_maps = []
    for core in range(NCORES):
        b = core // 2
        h0 = HPC * (core % 2)
        cols = slice(h0 * HD, (h0 + HPC) * HD)
        w3 = np.ascontiguousarray(
            np.concatenate([Wq[:, cols], Wk[:, cols], Wv[:, cols]], axis=1)
        ).astype(ml_dtypes.bfloat16)
        b3 = np.concatenate([bq[cols], bk[cols], bv[cols]])
        # scores get multiplied by 1/8 inside exp; pre-scale mask add by 8 so
        # the net additive term matches the reference's -10000*(1-mask).
        maskrow = ((1.0 - attention_mask[b]) * (-80000.0)).astype(ml_dtypes.bfloat16)
        in_maps.append(
            {
                "hs": np.ascontiguousarray(hidden_states[b]).astype(ml_dtypes.bfloat16),
                "w3": w3,
                "b3": np.ascontiguousarray(b3),
                "maskrow": maskrow,
            }
        )
    return in_maps


def run(in_maps, **kwargs):
    from concourse.bass_utils import run_bass_kernel_spmd

    nc = _get_nc()
    return run_bass_kernel_spmd(nc, in_maps, core_ids=list(range(NCORES)), **kwargs)


def kernel(hidden_states, attention_mask, Wq, bq, Wk, bk, Wv, bv):
    in_maps = make_in_maps(hidden_states, attention_mask, Wq, bq, Wk, bk, Wv, bv)
    res = run(in_maps)
    ctx = np.empty((B, S, D), dtype=np.float32)
    probs = np.empty((B, H, S, S), dtype=np.float32)
    for core in range(NCORES):
        b = core // 2
        h0 = HPC * (core % 2)
        r = res.results[core]
        probs[b, h0 : h0 + HPC] = r["probs_out"]
        ctx[b, :, h0 * HD : (h0 + HPC) * HD] = r["ctx_out"]
    return ctx, probs


if __name__ == "__main__":
    nc = build_bass()
    print("built ok")
